# revision 5
# baseline (speedup 1.0000x reference)
"""LoRA MultiheadAttention on 8 Trainium2 NeuronCores (Bass/Tile).

Sharding: core c = (batch n = c//2, head-group hg = c%2); each core handles
6 of 12 heads for one of 4 batches. LoRA is folded into the projection
weights on the host (W_eff = W + scale * up @ down — mathematically
identical). Inputs are shipped pre-transposed (E-major) per shard. Each core
computes q^T/k^T (E-major), v (S-major, with a ones column per head for the
softmax denominator), full-softmax attention in fp16 with fp32 accumulation,
and a half-K out-projection partial. The host sums the two partials per
batch and adds the output bias (pure unshard glue).
"""
import numpy as np

import concourse.bass as bass
import concourse.tile as tile
from concourse import bacc, mybir
from concourse.bass_utils import run_bass_kernel_spmd
from concourse.masks import make_identity

L, N, E, H, R = 2048, 4, 768, 12, 16
ALPHA = 16.0
LORA_SCALE = ALPHA / R
HD = E // H          # 64
HG = 2               # head groups (column-parallel dimension)
HPG = H // HG        # 6 heads per group
EG = E // HG         # 384 columns per group
NC_ = 8
F32 = mybir.dt.float32
F16 = mybir.dt.float16
SCALE = 1.0 / float(np.sqrt(HD))  # folded into exp's input scale

_CACHED = {}


def _build():
    nc = bacc.Bacc()
    # per-core external I/O (shapes are per-shard)
    xqT = nc.dram_tensor("xqT", [E, L], F32, kind="ExternalInput")
    xkT = nc.dram_tensor("xkT", [E, L], F32, kind="ExternalInput")
    xvT = nc.dram_tensor("xvT", [E, L], F32, kind="ExternalInput")
    wqT = nc.dram_tensor("wqT", [E, EG], F32, kind="ExternalInput")
    wkT = nc.dram_tensor("wkT", [E, EG], F32, kind="ExternalInput")
    wvT = nc.dram_tensor("wvT", [E, EG], F32, kind="ExternalInput")
    woT = nc.dram_tensor("woT", [EG, E], F32, kind="ExternalInput")
    bq = nc.dram_tensor("bq", [EG], F32, kind="ExternalInput")
    bk = nc.dram_tensor("bk", [EG], F32, kind="ExternalInput")
    bv = nc.dram_tensor("bv", [EG], F32, kind="ExternalInput")
    out = nc.dram_tensor("out", [L, E], F32, kind="ExternalOutput")

    KC = E // 128    # 6 contraction chunks
    EC = EG // 128   # 3 output chunks per projection
    LT = L // 128    # 16 l/s tiles
    VW = HPG * (HD + 1)  # 390: per-head 64 v cols + 1 ones col

    with tile.TileContext(nc) as tc:
        with (
            tc.tile_pool(name="stage", bufs=2) as stage,
            tc.tile_pool(name="big", bufs=18) as big,
            tc.tile_pool(name="persist", bufs=1) as persist,
            tc.tile_pool(name="small", bufs=4) as small,
            tc.tile_pool(name="outsb", bufs=3) as outsb_pool,
            tc.tile_pool(name="psum", bufs=1, space="PSUM") as psum,
        ):
            # ---- constants / weights ----
            ident = persist.tile([128, 128], F16, name="ident")
            make_identity(nc, ident)

            w16 = {}
            for pname, wdram in (("q", wqT), ("k", wkT), ("v", wvT)):
                for j in range(KC):
                    w32 = stage.tile([128, 2048], F32, tag="stage", name="w32")
                    nc.sync.dma_start(w32[:, :EG], wdram[j * 128:(j + 1) * 128, :])
                    wt = persist.tile([128, EG], F16, name=f"w16_{pname}{j}")
                    nc.vector.tensor_copy(wt[:], w32[:, :EG])
                    w16[pname, j] = wt
            wo16 = []
            for j in range(EC):
                w32 = stage.tile([128, 2048], F32, tag="stage", name="w32")
                nc.sync.dma_start(w32[:, :E], woT[j * 128:(j + 1) * 128, :])
                wt = persist.tile([128, E], F16, name=f"wo16_{j}")
                nc.vector.tensor_copy(wt[:], w32[:, :E])
                wo16.append(wt)

            bias_t = {}
            for bname, bdram in (("q", bq), ("k", bk), ("v", bv)):
                for j in range(EC):
                    bt = persist.tile([128, 1], F32, name=f"b_{bname}{j}")
                    nc.sync.dma_start(bt[:], bdram[j * 128:(j + 1) * 128])
                    bias_t[bname, j] = bt

            # ---- projections ----
            qkT = {}   # ("q"|"k", e-chunk) -> (128, L) f16, E-major
            v_aug = []  # 16 tiles (128, VW) f16, per-head [64 v | 1.0]
            for pname, xdram in (("q", xqT), ("k", xkT), ("v", xvT)):
                x16 = []
                for j in range(KC):
                    x32 = stage.tile([128, 2048], F32, tag="stage", name="x32")
                    nc.sync.dma_start(x32[:], xdram[j * 128:(j + 1) * 128, :])
                    xt = big.tile([128, L], F16, tag="big", name="x16")
                    nc.vector.tensor_copy(xt[:], x32[:])
                    x16.append(xt)
                if pname in ("q", "k"):
                    for e in range(EC):
                        dst = persist.tile([128, L], F16, name=f"{pname}T{e}")
                        qkT[pname, e] = dst
                        for lc in range(2):
                            mm = psum.tile([128, 1024], F32, tag="mm", bufs=2,
                                           name="mm_proj")
                            for half in range(2):
                                o_sl = mm[:, half * 512:(half + 1) * 512]
                                l0 = lc * 1024 + half * 512
                                for kk in range(KC):
                                    nc.tensor.matmul(
                                        o_sl,
                                        w16[pname, kk][:, e * 128:(e + 1) * 128],
                                        x16[kk][:, l0:l0 + 512],
                                        start=(kk == 0), stop=(kk == KC - 1),
                                    )
                            nc.vector.tensor_scalar_add(
                                dst[:, lc * 1024:(lc + 1) * 1024], mm[:],
                                bias_t[pname, e][:],
                            )
                else:
                    for st in range(LT):
                        mm = psum.tile([128, 1024], F32, tag="mm", bufs=2,
                                       name="mm_vproj")
                        for kk in range(KC):
                            nc.tensor.matmul(
                                mm[:, 0:EG],
                                x16[kk][:, st * 128:(st + 1) * 128],
                                w16["v", kk][:],
                                start=(kk == 0), stop=(kk == KC - 1),
                            )
                        vt = persist.tile([128, VW], F16, name=f"v_aug{st}")
                        grp = vt.rearrange("p (h c) -> p h c", c=HD + 1)
                        nc.vector.tensor_copy(
                            grp[:, :, 0:HD],
                            mm[:, 0:EG].rearrange("p (h c) -> p h c", c=HD),
                        )
                        nc.vector.memset(grp[:, :, HD:HD + 1], 1.0)
                        v_aug.append(vt)

            # ---- attention ----
            o_n = [persist.tile([128, EG], F16, name=f"o_n{lt}")
                   for lt in range(LT)]
            for h in range(HPG):
                et, pb = h // 2, (h % 2) * 64
                qs = qkT["q", et][pb:pb + 64, :]
                ks = qkT["k", et][pb:pb + 64, :]
                attn = []
                for st in range(LT):
                    at = big.tile([128, L], F16, tag="big", name="attn")
                    for lc in range(2):
                        sc = psum.tile([128, 1024], F32, tag="mm", bufs=2,
                                       name="mm_sc")
                        for half in range(2):
                            l0 = lc * 1024 + half * 512
                            nc.tensor.matmul(
                                sc[:, half * 512:(half + 1) * 512],
                                ks[:, st * 128:(st + 1) * 128],
                                qs[:, l0:l0 + 512],
                                start=True, stop=True,
                            )
                        nc.scalar.activation(
                            at[:, lc * 1024:(lc + 1) * 1024], sc[:],
                            mybir.ActivationFunctionType.Exp, scale=SCALE,
                        )
                    attn.append(at)
                for lt in range(LT):
                    oh = psum.tile([128, HD + 1], F32, tag="oh", bufs=2,
                                   name="oh")
                    for st in range(LT):
                        nc.tensor.matmul(
                            oh[:],
                            attn[st][:, lt * 128:(lt + 1) * 128],
                            v_aug[st][:, h * (HD + 1):(h + 1) * (HD + 1)],
                            start=(st == 0), stop=(st == LT - 1),
                        )
                    recip = small.tile([128, 1], F32, tag="recip", name="recip")
                    nc.vector.reciprocal(recip[:], oh[:, HD:HD + 1])
                    nc.vector.tensor_scalar_mul(
                        o_n[lt][:, h * HD:(h + 1) * HD], oh[:, 0:HD], recip[:],
                    )

            # ---- o^T (PE transpose) + bias_v, then out-projection ----
            oT = [persist.tile([128, L], F16, name=f"oT{j}") for j in range(EC)]
            for lt in range(LT):
                for j in range(EC):
                    tr = psum.tile([128, 128], F16, tag="tr", bufs=2, name="tr")
                    nc.tensor.transpose(
                        tr[:], o_n[lt][:, j * 128:(j + 1) * 128], ident[:])
                    nc.vector.tensor_scalar_add(
                        oT[j][:, lt * 128:(lt + 1) * 128], tr[:],
                        bias_t["v", j][:],
                    )
            for lt in range(LT):
                osb = outsb_pool.tile([128, E], F32, tag="osb", name="osb")
                for nh in range(2):
                    po = psum.tile([128, 1024], F32, tag="mm", bufs=2,
                                   name="mm_out")
                    for j in range(EC):
                        nc.tensor.matmul(
                            po[:, 0:EG],
                            oT[j][:, lt * 128:(lt + 1) * 128],
                            wo16[j][:, nh * EG:(nh + 1) * EG],
                            start=(j == 0), stop=(j == EC - 1),
                        )
                    nc.vector.tensor_copy(
                        osb[:, nh * EG:(nh + 1) * EG], po[:, 0:EG])
                nc.sync.dma_start(out[lt * 128:(lt + 1) * 128, :], osb[:])
    nc.finalize()
    return nc


def kernel(query, key, value, in_proj_weight, in_proj_bias,
           q_down, q_up, k_down, k_up, v_down, v_up,
           out_proj_weight, out_proj_bias, out_down, out_up):
    if "nc" not in _CACHED:
        _CACHED["nc"] = _build()
    nc = _CACHED["nc"]

    f = np.float32
    # fold LoRA into the projection weights (exact algebraic identity)
    w_eff = {}
    for i, (dn, up) in enumerate(((q_down, q_up), (k_down, k_up),
                                  (v_down, v_up))):
        w = in_proj_weight[i * E:(i + 1) * E].astype(f)
        w_eff[i] = w + LORA_SCALE * (up.astype(f) @ dn.astype(f))
    wo_eff = out_proj_weight.astype(f) + LORA_SCALE * (
        out_up.astype(f) @ out_down.astype(f))

    in_maps = []
    for c in range(NC_):
        n, hg = c // 2, c % 2
        sl = slice(hg * EG, (hg + 1) * EG)
        m = {
            "xqT": np.ascontiguousarray(query[:, n, :].T, dtype=f),
            "xkT": np.ascontiguousarray(key[:, n, :].T, dtype=f),
            "xvT": np.ascontiguousarray(value[:, n, :].T, dtype=f),
            "wqT": np.ascontiguousarray(w_eff[0][sl].T, dtype=f),
            "wkT": np.ascontiguousarray(w_eff[1][sl].T, dtype=f),
            "wvT": np.ascontiguousarray(w_eff[2][sl].T, dtype=f),
            "woT": np.ascontiguousarray(wo_eff[:, sl].T, dtype=f),
            "bq": np.ascontiguousarray(in_proj_bias[0:E][sl], dtype=f),
            "bk": np.ascontiguousarray(in_proj_bias[E:2 * E][sl], dtype=f),
            "bv": np.ascontiguousarray(in_proj_bias[2 * E:3 * E][sl], dtype=f),
        }
        in_maps.append(m)

    _CACHED["in_maps"] = in_maps
    res = run_bass_kernel_spmd(nc, in_maps, list(range(NC_)))
    outp = np.empty((L, N, E), dtype=np.float32)
    bo = out_proj_bias.astype(f)
    for n in range(N):
        outp[:, n, :] = (res.results[2 * n]["out"]
                         + res.results[2 * n + 1]["out"] + bo)
    return outp
